# revision 3
# baseline (speedup 1.0000x reference)
"""Trainium2 Bass kernel for block-sparse masked attention (nn_Attention_970662609463).

Model (reference.py):
    B=2, N=4096, D=1024, heads=16, dim_head=64, f=8 chunks of n=512 tokens.
    qkv = x @ w_qkv.T ; per (batch, head, chunk) block of 512 tokens:
    sim = scale * q k^T, diag + key-mask -> -inf, softmax, out = attn @ v,
    y = out @ w_out.T + b_out.

Sharding: 16 global (batch, chunk) token groups of 512 tokens; each of the 8
cores processes 2 of them for all 16 heads (qkv proj + attention + out proj are
all token-local because attention is block-diagonal in tokens). No collectives.

Per-core layout strategy (all matmuls bf16 with fp32 PSUM accumulation):
    - x^T, w_qkv^T resident in SBUF; q,k computed in [e, t] (transposed) layout,
      v in natural [t, e] layout with an extra ones column per head.
    - sim^T[j, i] blocks: per j-tile matmul, key mask applied as per-partition
      bias inside the exp activation; diagonal masked via a (1-eye) multiply.
    - attn@v and the softmax denominator come from one PSUM matmul group
      (the ones column makes row 64 the per-i sum of masked exp).
    - normalization by 1/sum via reciprocal + gpsimd partition broadcast.
    - final projection back to natural [t, d] layout, fp32 out.
"""

import os
import threading

import numpy as np
import ml_dtypes

B, N, D = 2, 4096, 1024
HEADS, DH = 16, 64
F, NB = 8, 512            # chunks per batch row, tokens per chunk
INNER = HEADS * DH        # 1024
E3 = 3 * INNER            # 3072
NCORES = 8
CPC = 2                   # chunks per core
TPC = CPC * NB            # tokens per core
KT = D // 128             # k-tiles over the model dim
MASK_NEG = -30000.0       # exp(scale*sim + MASK_NEG) == 0.0 in fp32
SCALE = DH ** -0.5

BF16NP = ml_dtypes.bfloat16


def _build_bass():
    import concourse.bacc as bacc
    import concourse.tile as tile
    import concourse.mybir as mybir
    from contextlib import ExitStack

    BF16 = mybir.dt.bfloat16
    F32 = mybir.dt.float32
    EXP = mybir.ActivationFunctionType.Exp

    nc = bacc.Bacc(trn_type="TRN2", debug=False)

    xT = nc.dram_tensor("xT", [D, TPC], BF16, kind="ExternalInput").ap()
    wqkvT = nc.dram_tensor("wqkvT", [D, E3], BF16, kind="ExternalInput").ap()
    woutT = nc.dram_tensor("woutT", [INNER, D], BF16, kind="ExternalInput").ap()
    maskT = nc.dram_tensor("maskT", [128, CPC, HEADS, 4], F32, kind="ExternalInput").ap()
    eyec = nc.dram_tensor("eyec", [128, 4, NB], BF16, kind="ExternalInput").ap()
    y = nc.dram_tensor("y", [TPC, D], F32, kind="ExternalOutput").ap()

    with tile.TileContext(nc) as tc, ExitStack() as ctx:
        persist = ctx.enter_context(tc.tile_pool(name="persist", bufs=1))
        qkpool = ctx.enter_context(tc.tile_pool(name="qkp", bufs=2))
        vapool = ctx.enter_context(tc.tile_pool(name="vap", bufs=2))
        epool = ctx.enter_context(tc.tile_pool(name="epool", bufs=8))
        opool = ctx.enter_context(tc.tile_pool(name="opool", bufs=2))
        ypool = ctx.enter_context(tc.tile_pool(name="ypool", bufs=2))
        spool = ctx.enter_context(tc.tile_pool(name="spool", bufs=4))
        qkv_ps = ctx.enter_context(tc.tile_pool(name="qkvps", bufs=2, space="PSUM"))
        sim_ps = ctx.enter_context(tc.tile_pool(name="simps", bufs=4, space="PSUM"))
        av_ps = ctx.enter_context(tc.tile_pool(name="avps", bufs=2, space="PSUM"))

        # Persistent loads
        w_sb, x_sb, wo_sb = [], [], []
        for k in range(KT):
            w = persist.tile([128, E3], BF16, name=f"w{k}", tag=f"w{k}")
            nc.sync.dma_start(out=w, in_=wqkvT[k * 128:(k + 1) * 128, :])
            w_sb.append(w)
            xs = persist.tile([128, TPC], BF16, name=f"x{k}", tag=f"x{k}")
            nc.sync.dma_start(out=xs, in_=xT[k * 128:(k + 1) * 128, :])
            x_sb.append(xs)
            wo = persist.tile([128, D], BF16, name=f"wo{k}", tag=f"wo{k}")
            nc.sync.dma_start(out=wo, in_=woutT[k * 128:(k + 1) * 128, :])
            wo_sb.append(wo)
        ec_sb = persist.tile([128, 4, NB], BF16, name="ec", tag="ec")
        nc.sync.dma_start(out=ec_sb, in_=eyec)
        mb_sb = persist.tile([128, CPC, HEADS, 4], F32, name="mb", tag="mb")
        nc.sync.dma_start(out=mb_sb, in_=maskT)

        for c in range(CPC):
            tok = slice(c * NB, (c + 1) * NB)

            # ---- q/k projection into transposed [e, t] tiles ----
            # e-tiles 0..7 are q (heads 2m, 2m+1), 8..15 are k.
            qk_sb = []
            for m in range(16):
                ps = qkv_ps.tile([128, NB], F32, name="qkvps", tag="qkvps")
                for k in range(KT):
                    nc.tensor.matmul(
                        ps,
                        lhsT=w_sb[k][:, m * 128:(m + 1) * 128],
                        rhs=x_sb[k][:, tok],
                        start=(k == 0),
                        stop=(k == KT - 1),
                    )
                t = qkpool.tile([128, NB], BF16, name=f"qk{m}", tag=f"qk{m}")
                nc.scalar.copy(out=t, in_=ps)
                qk_sb.append(t)

            # ---- v projection, natural [t, e] layout + ones column per head ----
            va_sb = []
            for tt in range(4):
                va = vapool.tile([128, HEADS, DH + 1], BF16, name=f"va{tt}", tag=f"va{tt}")
                nc.vector.memset(va[:, :, DH:DH + 1], 1.0)
                for half in range(2):
                    ps = qkv_ps.tile([128, NB], F32, name="vps", tag="qkvps")
                    for k in range(KT):
                        nc.tensor.matmul(
                            ps,
                            lhsT=x_sb[k][:, c * NB + tt * 128:c * NB + (tt + 1) * 128],
                            rhs=w_sb[k][:, 2 * INNER + half * NB:2 * INNER + (half + 1) * NB],
                            start=(k == 0),
                            stop=(k == KT - 1),
                        )
                    nc.scalar.copy(
                        out=va[:, half * 8:(half + 1) * 8, 0:DH],
                        in_=ps.rearrange("p (g d) -> p g d", d=DH),
                    )
                va_sb.append(va)

            # ---- per-head attention ----
            o_sb = [
                opool.tile([128, NB], BF16, name=f"o{m}", tag=f"o{m}")
                for m in range(8)
            ]
            for h in range(HEADS):
                mt, off = h // 2, (h % 2) * 64
                q_ap = qk_sb[h // 2][off:off + 64, :]
                Es = []
                for jt in range(4):
                    sps = sim_ps.tile([128, NB], F32, name="sps", tag="sps")
                    nc.tensor.matmul(
                        sps,
                        lhsT=qk_sb[8 + h // 2][off:off + 64, jt * 128:(jt + 1) * 128],
                        rhs=q_ap,
                        start=True,
                        stop=True,
                    )
                    Ee = epool.tile([128, NB], BF16, name="Ee", tag="Ee")
                    nc.scalar.activation(
                        out=Ee, in_=sps, func=EXP,
                        bias=mb_sb[:, c, h, jt:jt + 1], scale=SCALE,
                    )
                    nc.vector.tensor_mul(out=Ee, in0=Ee, in1=ec_sb[:, jt, :])
                    Es.append(Ee)

                avp = av_ps.tile([128, NB], F32, name="avp", tag="avp")
                for jt in range(4):
                    nc.tensor.matmul(
                        avp[0:DH + 1, :],
                        lhsT=va_sb[jt][:, h, :],
                        rhs=Es[jt],
                        start=(jt == 0),
                        stop=(jt == 3),
                    )
                rs = spool.tile([1, NB], F32, name="rs", tag="rs")
                nc.vector.reciprocal(out=rs, in_=avp[DH:DH + 1, :])
                bc = spool.tile([64, NB], F32, name="bc", tag="bc")
                nc.gpsimd.partition_broadcast(bc, rs)
                if off == 0:
                    nc.vector.tensor_mul(out=o_sb[mt][0:64, :], in0=avp[0:DH, :], in1=bc)
                else:
                    # DVE lanes cannot shift partitions; bounce through SBUF DMA.
                    tmp = spool.tile([64, NB], BF16, name="tmp", tag="tmp")
                    nc.vector.tensor_mul(out=tmp, in0=avp[0:DH, :], in1=bc)
                    nc.sync.dma_start(out=o_sb[mt][64:128, :], in_=tmp)

            # ---- output projection back to natural [t, d] ----
            for tt in range(4):
                yb = ypool.tile([128, D], F32, name="yb", tag="yb")
                for half in range(2):
                    fps = qkv_ps.tile([128, NB], F32, name="fps", tag="qkvps")
                    for mt in range(8):
                        nc.tensor.matmul(
                            fps,
                            lhsT=o_sb[mt][:, tt * 128:(tt + 1) * 128],
                            rhs=wo_sb[mt][:, half * NB:(half + 1) * NB],
                            start=(mt == 0),
                            stop=(mt == 7),
                        )
                    nc.scalar.copy(out=yb[:, half * NB:(half + 1) * NB], in_=fps)
                nc.sync.dma_start(
                    out=y[c * NB + tt * 128:c * NB + (tt + 1) * 128, :], in_=yb
                )

    nc.compile()
    return nc


_cache = threading.Lock()
_built = {}


def get_bass():
    with _cache:
        if "nc" not in _built:
            _built["nc"] = _build_bass()
        return _built["nc"]


def make_in_maps(x, w_qkv, w_out, mask):
    """Build the 8 per-core input dicts from full inputs."""
    x = np.asarray(x, dtype=np.float32)
    w_qkv = np.asarray(w_qkv, dtype=np.float32)
    w_out = np.asarray(w_out, dtype=np.float32)
    mask = np.asarray(mask)

    wqkvT = np.ascontiguousarray(w_qkv.T).astype(BF16NP)      # [D, 3*inner]
    woutT = np.ascontiguousarray(w_out.T).astype(BF16NP)      # [inner, D]

    # (1 - eye) tiles in the sim^T [j, i] layout: ec[p, jt, i] = 0 iff jt*128+p == i
    jidx = (np.arange(4)[:, None] * 128 + np.arange(128)[None, :])  # [jt, p] -> j
    ec = np.ones((128, 4, NB), np.float32)
    for jt in range(4):
        ec[np.arange(128), jt, jidx[jt]] = 0.0
    ec = ec.astype(BF16NP)

    xr = x.reshape(B, F, NB, D)
    maskr = mask.reshape(B, HEADS, F, NB)

    in_maps = []
    for core in range(NCORES):
        chunks = (2 * core, 2 * core + 1)
        xc = np.concatenate([xr[g // F, g % F] for g in chunks], axis=0)  # [TPC, D]
        xT = np.ascontiguousarray(xc.T).astype(BF16NP)                    # [D, TPC]
        mb = np.zeros((CPC, HEADS, 4, 128), np.float32)
        for ci, g in enumerate(chunks):
            mrow = maskr[g // F, :, g % F, :]                             # [HEADS, NB]
            mb[ci] = np.where(mrow.reshape(HEADS, 4, 128) == 0, MASK_NEG, 0.0)
        maskT = np.ascontiguousarray(mb.transpose(3, 0, 1, 2))            # [128, CPC, HEADS, 4]
        in_maps.append({
            "xT": xT,
            "wqkvT": wqkvT,
            "woutT": woutT,
            "maskT": maskT,
            "eyec": ec,
        })
    return in_maps


def assemble_output(results, b_out):
    """Stitch the 8 per-core [TPC, D] outputs back into [B, N, D]."""
    y = np.empty((B, N, D), np.float32)
    for core in range(NCORES):
        yc = results[core]["y"]
        for ci, g in enumerate((2 * core, 2 * core + 1)):
            b, fi = g // F, g % F
            y[b, fi * NB:(fi + 1) * NB, :] = yc[ci * NB:(ci + 1) * NB, :]
    y += np.asarray(b_out, dtype=np.float32)[None, None, :]
    return y


def run(x, w_qkv, w_out, b_out, mask, trace=False, **spmd_kwargs):
    from concourse.bass_utils import run_bass_kernel_spmd

    nc = get_bass()
    in_maps = make_in_maps(x, w_qkv, w_out, mask)
    res = run_bass_kernel_spmd(
        nc, in_maps, core_ids=list(range(NCORES)), trace=trace, **spmd_kwargs
    )
    return assemble_output(res.results, b_out), res


def kernel(x, w_qkv, w_out, b_out, mask, f, diag):
    assert int(f) == F and int(diag) == 1, (f, diag)
    out, _ = run(x, w_qkv, w_out, b_out, mask)
    return out


# revision 11
# speedup vs baseline: 118.9175x; 118.9175x over previous
"""Trainium2 Bass kernel for block-sparse masked attention (nn_Attention_970662609463).

Model (reference.py):
    B=2, N=4096, D=1024, heads=16, dim_head=64, f=8 chunks of n=512 tokens.
    qkv = x @ w_qkv.T ; per (batch, head, chunk) block of 512 tokens:
    sim = scale * q k^T, diag + key-mask -> -inf, softmax, out = attn @ v,
    y = out @ w_out.T + b_out.

Sharding: 16 global (batch, chunk) token groups of 512 tokens; each of the 8
cores processes 2 of them for all 16 heads (qkv proj + attention + out proj are
all token-local because attention is block-diagonal in tokens). No collectives.

Per-core layout strategy (all matmuls bf16 with fp32 PSUM accumulation):
    - x^T, w_qkv^T resident in SBUF; q,k computed in [e, t] (transposed) layout,
      v in natural [t, e] layout with an extra ones column per head.
    - sim^T[j, i] blocks: per j-tile matmul, key mask applied as per-partition
      bias inside the exp activation; diagonal masked via a (1-eye) multiply.
    - attn@v and the softmax denominator come from one PSUM matmul group
      (the ones column makes row 64 the per-i sum of masked exp).
    - normalization by 1/sum via reciprocal + gpsimd partition broadcast.
    - final projection back to natural [t, d] layout, fp32 out.
"""

import os
import threading

import numpy as np
import ml_dtypes

B, N, D = 2, 4096, 1024
HEADS, DH = 16, 64
F, NB = 8, 512            # chunks per batch row, tokens per chunk
INNER = HEADS * DH        # 1024
E3 = 3 * INNER            # 3072
NCORES = 8
CPC = 2                   # chunks per core
TPC = CPC * NB            # tokens per core
KT = D // 128             # k-tiles over the model dim
MASK_NEG = -30000.0       # exp(scale*sim + MASK_NEG) == 0.0 in fp32
SCALE = DH ** -0.5

BF16NP = ml_dtypes.bfloat16


def _build_bass(repeat=1, loop=1):
    """Build the per-core program. repeat>1 re-emits the whole body (loads
    included) that many times; loop>1 wraps it in a hardware For_i loop —
    both used only for slope-based wall-clock timing."""
    import concourse.bacc as bacc
    import concourse.tile as tile
    import concourse.mybir as mybir
    from contextlib import ExitStack, nullcontext

    BF16 = mybir.dt.bfloat16
    F32 = mybir.dt.float32
    EXP = mybir.ActivationFunctionType.Exp

    nc = bacc.Bacc(trn_type="TRN2", debug=False)

    xT = nc.dram_tensor("xT", [D, TPC], BF16, kind="ExternalInput").ap()
    wqkvT = nc.dram_tensor("wqkvT", [D, E3], BF16, kind="ExternalInput").ap()
    woutT = nc.dram_tensor("woutT", [INNER, D], BF16, kind="ExternalInput").ap()
    maskT = nc.dram_tensor("maskT", [128, CPC, HEADS, 4], F32, kind="ExternalInput").ap()
    eyec = nc.dram_tensor("eyec", [128, 4, NB], BF16, kind="ExternalInput").ap()
    y = nc.dram_tensor("y", [TPC, D], F32, kind="ExternalOutput").ap()

    with tile.TileContext(nc) as tc, ExitStack() as ctx:
        persist = ctx.enter_context(tc.tile_pool(name="persist", bufs=1))
        qkpool = ctx.enter_context(tc.tile_pool(name="qkp", bufs=2))
        vapool = ctx.enter_context(tc.tile_pool(name="vap", bufs=2))
        epool = ctx.enter_context(tc.tile_pool(name="epool", bufs=8))
        opool = ctx.enter_context(tc.tile_pool(name="opool", bufs=2))
        ypool = ctx.enter_context(tc.tile_pool(name="ypool", bufs=2))
        spool = ctx.enter_context(tc.tile_pool(name="spool", bufs=4))
        # 8 PSUM banks: qkv 2 + sim 4 + av/final 2
        qkv_ps = ctx.enter_context(tc.tile_pool(name="qkvps", bufs=2, space="PSUM"))
        sim_ps = ctx.enter_context(tc.tile_pool(name="simps", bufs=4, space="PSUM"))
        av_ps = ctx.enter_context(tc.tile_pool(name="avps", bufs=2, space="PSUM"))

        loop_cm = tc.For_i(0, loop, 1) if loop > 1 else nullcontext()
        ctx.enter_context(loop_cm)
        for _rep in range(repeat):
            # Loads, split and ordered so the first matmuls can start as soon
            # as their slices land (subtile deps track per-column ranges).
            mb_sb = persist.tile([128, CPC, HEADS, 4], F32, name="mb", tag="mb")
            nc.sync.dma_start(out=mb_sb, in_=maskT)
            ec_sb = persist.tile([128, 4, NB], BF16, name="ec", tag="ec")
            nc.sync.dma_start(out=ec_sb, in_=eyec)

            w_sb = [persist.tile([128, E3], BF16, name=f"w{k}", tag=f"w{k}") for k in range(KT)]
            x_sb = [persist.tile([128, TPC], BF16, name=f"x{k}", tag=f"x{k}") for k in range(KT)]
            wo_sb = [persist.tile([128, D], BF16, name=f"wo{k}", tag=f"wo{k}") for k in range(KT)]
            for k in range(KT):  # x chunk 0 first: every qkv matmul reads it
                nc.sync.dma_start(out=x_sb[k][:, 0:NB], in_=xT[k * 128:(k + 1) * 128, 0:NB])
            for part in range(6):  # w_qkv in 512-col slices: q cols, k cols, v cols
                lo, hi = part * 512, (part + 1) * 512
                for k in range(KT):
                    nc.sync.dma_start(
                        out=w_sb[k][:, lo:hi], in_=wqkvT[k * 128:(k + 1) * 128, lo:hi]
                    )
            for k in range(KT):
                nc.sync.dma_start(
                    out=x_sb[k][:, NB:TPC], in_=xT[k * 128:(k + 1) * 128, NB:TPC]
                )
            for k in range(KT):
                nc.sync.dma_start(out=wo_sb[k], in_=woutT[k * 128:(k + 1) * 128, :])

            for c in range(CPC):
                tok = slice(c * NB, (c + 1) * NB)

                # ---- q/k projection into transposed [e, t] tiles ----
                # e-tiles 0..7 are q (heads 2m, 2m+1), 8..15 are k.
                qk_sb = []
                for m in range(16):
                    ps = qkv_ps.tile([128, NB], F32, name="qkvps", tag="qkvps")
                    for k in range(KT):
                        nc.tensor.matmul(
                            ps,
                            lhsT=w_sb[k][:, m * 128:(m + 1) * 128],
                            rhs=x_sb[k][:, tok],
                            start=(k == 0),
                            stop=(k == KT - 1),
                        )
                    t = qkpool.tile([128, NB], BF16, name=f"qk{m}", tag=f"qk{m}")
                    nc.vector.tensor_copy(out=t, in_=ps)
                    qk_sb.append(t)

                # ---- v projection, natural [t, e] layout + ones column per head ----
                va_sb = []
                for tt in range(4):
                    va = vapool.tile([128, HEADS, DH + 1], BF16, name=f"va{tt}", tag=f"va{tt}")
                    nc.vector.memset(va[:, :, DH:DH + 1], 1.0)
                    for half in range(2):
                        ps = qkv_ps.tile([128, NB], F32, name="vps", tag="qkvps")
                        for k in range(KT):
                            nc.tensor.matmul(
                                ps,
                                lhsT=x_sb[k][:, c * NB + tt * 128:c * NB + (tt + 1) * 128],
                                rhs=w_sb[k][:, 2 * INNER + half * NB:2 * INNER + (half + 1) * NB],
                                start=(k == 0),
                                stop=(k == KT - 1),
                            )
                        nc.scalar.copy(
                            out=va[:, half * 8:(half + 1) * 8, 0:DH],
                            in_=ps.rearrange("p (g d) -> p g d", d=DH),
                        )
                    va_sb.append(va)

                # ---- attention, one head pair (even at rows 0-63, odd at
                # 64-127) at a time: the two K=64 sim matmuls of a pair hit
                # disjoint PE row groups and run concurrently.
                o_sb = [
                    opool.tile([128, NB], BF16, name=f"o{m}", tag=f"o{m}")
                    for m in range(8)
                ]
                for mt in range(8):
                    Es = {0: [], 1: []}
                    for jt in range(4):
                        for par in (0, 1):
                            h, off = 2 * mt + par, par * 64
                            sps = sim_ps.tile([128, NB], F32, name="sps", tag="sps")
                            nc.tensor.matmul(
                                sps,
                                lhsT=qk_sb[8 + mt][off:off + 64, jt * 128:(jt + 1) * 128],
                                rhs=qk_sb[mt][off:off + 64, :],
                                start=True,
                                stop=True,
                            )
                            Ee = epool.tile([128, NB], BF16, name="Ee", tag="Ee")
                            nc.scalar.activation(
                                out=Ee, in_=sps, func=EXP,
                                bias=mb_sb[:, c, h, jt:jt + 1], scale=SCALE,
                            )
                            nc.vector.tensor_mul(out=Ee, in0=Ee, in1=ec_sb[:, jt, :])
                            Es[par].append(Ee)

                    for par in (0, 1):
                        h, off = 2 * mt + par, par * 64
                        avp = av_ps.tile([128, NB], F32, name="avp", tag="avp")
                        for jt in range(4):
                            nc.tensor.matmul(
                                avp[0:DH + 1, :],
                                lhsT=va_sb[jt][:, h, :],
                                rhs=Es[par][jt],
                                start=(jt == 0),
                                stop=(jt == 3),
                            )
                        rs = spool.tile([1, NB], F32, name="rs", tag="rs")
                        nc.vector.reciprocal(out=rs, in_=avp[DH:DH + 1, :])
                        bc = spool.tile([64, NB], F32, name="bc", tag="bc")
                        nc.gpsimd.partition_broadcast(bc, rs)
                        if off == 0:
                            nc.vector.tensor_mul(
                                out=o_sb[mt][0:64, :], in0=avp[0:DH, :], in1=bc
                            )
                        else:
                            # DVE lanes cannot shift partitions; bounce via DMA.
                            tmp = spool.tile([64, NB], BF16, name="tmp", tag="tmp")
                            nc.vector.tensor_mul(out=tmp, in0=avp[0:DH, :], in1=bc)
                            nc.sync.dma_start(out=o_sb[mt][64:128, :], in_=tmp)

                # ---- output projection back to natural [t, d] ----
                for tt in range(4):
                    yb = ypool.tile([128, D], F32, name="yb", tag="yb")
                    for half in range(2):
                        fps = av_ps.tile([128, NB], F32, name="fps", tag="avp")
                        for mt in range(8):
                            nc.tensor.matmul(
                                fps,
                                lhsT=o_sb[mt][:, tt * 128:(tt + 1) * 128],
                                rhs=wo_sb[mt][:, half * NB:(half + 1) * NB],
                                start=(mt == 0),
                                stop=(mt == 7),
                            )
                        nc.scalar.copy(out=yb[:, half * NB:(half + 1) * NB], in_=fps)
                    nc.sync.dma_start(
                        out=y[c * NB + tt * 128:c * NB + (tt + 1) * 128, :], in_=yb
                    )

    nc.compile()
    return nc


_cache = threading.Lock()
_built = {}


def get_bass(repeat=1, loop=1):
    with _cache:
        key = (repeat, loop)
        if key not in _built:
            _built[key] = _build_bass(repeat, loop)
        return _built[key]


def make_in_maps(x, w_qkv, w_out, mask):
    """Build the 8 per-core input dicts from full inputs."""
    x = np.asarray(x, dtype=np.float32)
    w_qkv = np.asarray(w_qkv, dtype=np.float32)
    w_out = np.asarray(w_out, dtype=np.float32)
    mask = np.asarray(mask)

    wqkvT = np.ascontiguousarray(w_qkv.T).astype(BF16NP)      # [D, 3*inner]
    woutT = np.ascontiguousarray(w_out.T).astype(BF16NP)      # [inner, D]

    # (1 - eye) tiles in the sim^T [j, i] layout: ec[p, jt, i] = 0 iff jt*128+p == i
    jidx = (np.arange(4)[:, None] * 128 + np.arange(128)[None, :])  # [jt, p] -> j
    ec = np.ones((128, 4, NB), np.float32)
    for jt in range(4):
        ec[np.arange(128), jt, jidx[jt]] = 0.0
    ec = ec.astype(BF16NP)

    xr = x.reshape(B, F, NB, D)
    maskr = mask.reshape(B, HEADS, F, NB)

    in_maps = []
    for core in range(NCORES):
        chunks = (2 * core, 2 * core + 1)
        xc = np.concatenate([xr[g // F, g % F] for g in chunks], axis=0)  # [TPC, D]
        xT = np.ascontiguousarray(xc.T).astype(BF16NP)                    # [D, TPC]
        mb = np.zeros((CPC, HEADS, 4, 128), np.float32)
        for ci, g in enumerate(chunks):
            mrow = maskr[g // F, :, g % F, :]                             # [HEADS, NB]
            mb[ci] = np.where(mrow.reshape(HEADS, 4, 128) == 0, MASK_NEG, 0.0)
        maskT = np.ascontiguousarray(mb.transpose(3, 0, 1, 2))            # [128, CPC, HEADS, 4]
        in_maps.append({
            "xT": xT,
            "wqkvT": wqkvT,
            "woutT": woutT,
            "maskT": maskT,
            "eyec": ec,
        })
    return in_maps


def assemble_output(results, b_out):
    """Stitch the 8 per-core [TPC, D] outputs back into [B, N, D]."""
    y = np.empty((B, N, D), np.float32)
    for core in range(NCORES):
        yc = results[core]["y"]
        for ci, g in enumerate((2 * core, 2 * core + 1)):
            b, fi = g // F, g % F
            y[b, fi * NB:(fi + 1) * NB, :] = yc[ci * NB:(ci + 1) * NB, :]
    y += np.asarray(b_out, dtype=np.float32)[None, None, :]
    return y


def run(x, w_qkv, w_out, b_out, mask, trace=False, **spmd_kwargs):
    from concourse.bass_utils import run_bass_kernel_spmd

    nc = get_bass()
    in_maps = make_in_maps(x, w_qkv, w_out, mask)
    res = run_bass_kernel_spmd(
        nc, in_maps, core_ids=list(range(NCORES)), trace=trace, **spmd_kwargs
    )
    return assemble_output(res.results, b_out), res


def kernel(x, w_qkv, w_out, b_out, mask, f, diag):
    assert int(f) == F and int(diag) == 1, (f, diag)
    out, _ = run(x, w_qkv, w_out, b_out, mask)
    return out


# revision 20
# speedup vs baseline: 125.7973x; 1.0579x over previous
"""Trainium2 Bass kernel for block-sparse masked attention (nn_Attention_970662609463).

Model (reference.py):
    B=2, N=4096, D=1024, heads=16, dim_head=64, f=8 chunks of n=512 tokens.
    qkv = x @ w_qkv.T ; per (batch, head, chunk) block of 512 tokens:
    sim = scale * q k^T, diag + key-mask -> -inf, softmax, out = attn @ v,
    y = out @ w_out.T + b_out.

Sharding: 16 global (batch, chunk) token groups of 512 tokens; each of the 8
cores processes 2 of them for all 16 heads (qkv proj + attention + out proj are
all token-local because attention is block-diagonal in tokens). No collectives.

Per-core layout strategy (all matmuls bf16 with fp32 PSUM accumulation):
    - x^T, w_qkv^T resident in SBUF; q,k computed in [e, t] (transposed) layout,
      v in natural [t, e] layout with an extra ones column per head.
    - sim^T[j, i] blocks: per j-tile matmul, key mask applied as per-partition
      bias inside the exp activation; diagonal masked via a (1-eye) multiply.
    - attn@v and the softmax denominator come from one PSUM matmul group
      (the ones column makes row 64 the per-i sum of masked exp).
    - normalization by 1/sum via reciprocal + gpsimd partition broadcast.
    - final projection back to natural [t, d] layout, fp32 out.
"""

import os
import threading

import numpy as np
import ml_dtypes

B, N, D = 2, 4096, 1024
HEADS, DH = 16, 64
F, NB = 8, 512            # chunks per batch row, tokens per chunk
INNER = HEADS * DH        # 1024
E3 = 3 * INNER            # 3072
NCORES = 8
CPC = 2                   # chunks per core
TPC = CPC * NB            # tokens per core
KT = D // 128             # k-tiles over the model dim
MASK_NEG = -30000.0       # exp(scale*sim + MASK_NEG) == 0.0 in fp32
SCALE = DH ** -0.5

BF16NP = ml_dtypes.bfloat16


def _build_bass(repeat=1, loop=1, loads_in_loop=True, pair=False):
    """Build the per-core program. repeat>1 re-emits the whole body (loads
    included) that many times; loop>1 wraps it in a hardware For_i loop —
    both used only for slope-based wall-clock timing. loads_in_loop=False
    hoists the input DMAs out of the timing loop (steady-state compute).
    pair=True emits consecutive matmuls sharing one stationary operand
    (both chunks per weight tile) to amortize PE weight loads."""
    import concourse.bacc as bacc
    import concourse.tile as tile
    import concourse.mybir as mybir
    from contextlib import ExitStack, nullcontext

    BF16 = mybir.dt.bfloat16
    F32 = mybir.dt.float32
    EXP = mybir.ActivationFunctionType.Exp

    nc = bacc.Bacc(trn_type="TRN2", debug=False)

    xT = nc.dram_tensor("xT", [D, TPC], BF16, kind="ExternalInput").ap()
    wqkvT = nc.dram_tensor("wqkvT", [D, E3], BF16, kind="ExternalInput").ap()
    woutT = nc.dram_tensor("woutT", [INNER, D], BF16, kind="ExternalInput").ap()
    maskT = nc.dram_tensor("maskT", [128, CPC, HEADS, 4], F32, kind="ExternalInput").ap()
    eyec = nc.dram_tensor("eyec", [128, 4, NB], BF16, kind="ExternalInput").ap()
    y = nc.dram_tensor("y", [TPC, D], F32, kind="ExternalOutput").ap()

    with tile.TileContext(nc) as tc, ExitStack() as ctx:
        persist = ctx.enter_context(tc.tile_pool(name="persist", bufs=1))
        qkpool = ctx.enter_context(tc.tile_pool(name="qkp", bufs=2))
        vapool = ctx.enter_context(tc.tile_pool(name="vap", bufs=2))
        epool = ctx.enter_context(tc.tile_pool(name="epool", bufs=8))
        opool = ctx.enter_context(tc.tile_pool(name="opool", bufs=2))
        ypool = ctx.enter_context(tc.tile_pool(name="ypool", bufs=2))
        spool = ctx.enter_context(tc.tile_pool(name="spool", bufs=4))
        # 8 PSUM banks: qkv + sim + av/final
        qkv_ps = ctx.enter_context(
            tc.tile_pool(name="qkvps", bufs=3 if pair else 2, space="PSUM")
        )
        sim_ps = ctx.enter_context(
            tc.tile_pool(name="simps", bufs=3 if pair else 4, space="PSUM")
        )
        av_ps = ctx.enter_context(tc.tile_pool(name="avps", bufs=2, space="PSUM"))

        def emit_loads():
            # Loads, split and ordered so the first matmuls can start as soon
            # as their slices land (subtile deps track per-column ranges).
            mb_sb = persist.tile([128, CPC, HEADS, 4], F32, name="mb", tag="mb")
            nc.sync.dma_start(out=mb_sb, in_=maskT)
            ec_sb = persist.tile([128, 4, NB], BF16, name="ec", tag="ec")
            nc.sync.dma_start(out=ec_sb, in_=eyec)

            w_sb = [persist.tile([128, E3], BF16, name=f"w{k}", tag=f"w{k}") for k in range(KT)]
            x_sb = [persist.tile([128, TPC], BF16, name=f"x{k}", tag=f"x{k}") for k in range(KT)]
            wo_sb = [persist.tile([128, D], BF16, name=f"wo{k}", tag=f"wo{k}") for k in range(KT)]
            for k in range(KT):  # x chunk 0 first: every qkv matmul reads it
                nc.sync.dma_start(out=x_sb[k][:, 0:NB], in_=xT[k * 128:(k + 1) * 128, 0:NB])
            if pair:  # paired qkv reads both chunks immediately
                for k in range(KT):
                    nc.sync.dma_start(
                        out=x_sb[k][:, NB:TPC], in_=xT[k * 128:(k + 1) * 128, NB:TPC]
                    )
            for part in range(6):  # w_qkv in 512-col slices: q cols, k cols, v cols
                lo, hi = part * 512, (part + 1) * 512
                for k in range(KT):
                    nc.sync.dma_start(
                        out=w_sb[k][:, lo:hi], in_=wqkvT[k * 128:(k + 1) * 128, lo:hi]
                    )
            if not pair:
                for k in range(KT):
                    nc.sync.dma_start(
                        out=x_sb[k][:, NB:TPC], in_=xT[k * 128:(k + 1) * 128, NB:TPC]
                    )
            for k in range(KT):
                nc.sync.dma_start(out=wo_sb[k], in_=woutT[k * 128:(k + 1) * 128, :])
            return mb_sb, ec_sb, w_sb, x_sb, wo_sb

        if not loads_in_loop:
            mb_sb, ec_sb, w_sb, x_sb, wo_sb = emit_loads()
        loop_cm = tc.For_i(0, loop, 1) if loop > 1 else nullcontext()
        ctx.enter_context(loop_cm)
        for _rep in range(repeat):
            if loads_in_loop:
                mb_sb, ec_sb, w_sb, x_sb, wo_sb = emit_loads()

            qk_all = {}   # (c, m) -> tile
            va_all = {}   # (c, tt) -> tile
            if pair:
                # Both chunks per stationary weight tile, back to back.
                for m in range(16):
                    ps2 = [
                        qkv_ps.tile([128, NB], F32, name="qkvps", tag="qkvps")
                        for _ in range(CPC)
                    ]
                    for k in range(KT):
                        for cc in range(CPC):
                            nc.tensor.matmul(
                                ps2[cc],
                                lhsT=w_sb[k][:, m * 128:(m + 1) * 128],
                                rhs=x_sb[k][:, cc * NB:(cc + 1) * NB],
                                start=(k == 0),
                                stop=(k == KT - 1),
                            )
                    for cc in range(CPC):
                        t = qkpool.tile([128, NB], BF16, name=f"qk{m}", tag=f"qk{m}")
                        nc.vector.tensor_copy(out=t, in_=ps2[cc])
                        qk_all[(cc, m)] = t
                for cc in range(CPC):
                    for tt in range(4):
                        va = vapool.tile(
                            [128, HEADS, DH + 1], BF16, name=f"va{tt}", tag=f"va{tt}"
                        )
                        nc.vector.memset(va[:, :, DH:DH + 1], 1.0)
                        ps2 = [
                            qkv_ps.tile([128, NB], F32, name="vps", tag="qkvps")
                            for _ in range(2)
                        ]
                        for k in range(KT):
                            for half in range(2):
                                nc.tensor.matmul(
                                    ps2[half],
                                    lhsT=x_sb[k][:, cc * NB + tt * 128:cc * NB + (tt + 1) * 128],
                                    rhs=w_sb[k][:, 2 * INNER + half * NB:2 * INNER + (half + 1) * NB],
                                    start=(k == 0),
                                    stop=(k == KT - 1),
                                )
                        for half in range(2):
                            nc.scalar.copy(
                                out=va[:, half * 8:(half + 1) * 8, 0:DH],
                                in_=ps2[half].rearrange("p (g d) -> p g d", d=DH),
                            )
                        va_all[(cc, tt)] = va

            for c in range(CPC):
                tok = slice(c * NB, (c + 1) * NB)

                if pair:
                    qk_sb = [qk_all[(c, m)] for m in range(16)]
                    va_sb = [va_all[(c, tt)] for tt in range(4)]
                else:
                    # ---- q/k projection into transposed [e, t] tiles ----
                    # e-tiles 0..7 are q (heads 2m, 2m+1), 8..15 are k.
                    qk_sb = []
                    for m in range(16):
                        ps = qkv_ps.tile([128, NB], F32, name="qkvps", tag="qkvps")
                        for k in range(KT):
                            nc.tensor.matmul(
                                ps,
                                lhsT=w_sb[k][:, m * 128:(m + 1) * 128],
                                rhs=x_sb[k][:, tok],
                                start=(k == 0),
                                stop=(k == KT - 1),
                            )
                        t = qkpool.tile([128, NB], BF16, name=f"qk{m}", tag=f"qk{m}")
                        nc.vector.tensor_copy(out=t, in_=ps)
                        qk_sb.append(t)

                    # ---- v projection, natural [t, e] + ones column per head ----
                    va_sb = []
                    for tt in range(4):
                        va = vapool.tile([128, HEADS, DH + 1], BF16, name=f"va{tt}", tag=f"va{tt}")
                        nc.vector.memset(va[:, :, DH:DH + 1], 1.0)
                        for half in range(2):
                            ps = qkv_ps.tile([128, NB], F32, name="vps", tag="qkvps")
                            for k in range(KT):
                                nc.tensor.matmul(
                                    ps,
                                    lhsT=x_sb[k][:, c * NB + tt * 128:c * NB + (tt + 1) * 128],
                                    rhs=w_sb[k][:, 2 * INNER + half * NB:2 * INNER + (half + 1) * NB],
                                    start=(k == 0),
                                    stop=(k == KT - 1),
                                )
                            nc.scalar.copy(
                                out=va[:, half * 8:(half + 1) * 8, 0:DH],
                                in_=ps.rearrange("p (g d) -> p g d", d=DH),
                            )
                        va_sb.append(va)

                # ---- attention, one head pair (even at rows 0-63, odd at
                # 64-127) at a time: the two K=64 sim matmuls of a pair hit
                # disjoint PE row groups and run concurrently.
                o_sb = [
                    opool.tile([128, NB], BF16, name=f"o{m}", tag=f"o{m}")
                    for m in range(8)
                ]
                for mt in range(8):
                    Es = {0: [], 1: []}
                    for jt in range(4):
                        for par in (0, 1):
                            h, off = 2 * mt + par, par * 64
                            sps = sim_ps.tile([128, NB], F32, name="sps", tag="sps")
                            nc.tensor.matmul(
                                sps,
                                lhsT=qk_sb[8 + mt][off:off + 64, jt * 128:(jt + 1) * 128],
                                rhs=qk_sb[mt][off:off + 64, :],
                                start=True,
                                stop=True,
                            )
                            Ee = epool.tile([128, NB], BF16, name="Ee", tag="Ee")
                            nc.scalar.activation(
                                out=Ee, in_=sps, func=EXP,
                                bias=mb_sb[:, c, h, jt:jt + 1], scale=SCALE,
                            )
                            nc.vector.tensor_mul(out=Ee, in0=Ee, in1=ec_sb[:, jt, :])
                            Es[par].append(Ee)

                    for par in (0, 1):
                        h, off = 2 * mt + par, par * 64
                        avp = av_ps.tile([128, NB], F32, name="avp", tag="avp")
                        for jt in range(4):
                            nc.tensor.matmul(
                                avp[0:DH + 1, :],
                                lhsT=va_sb[jt][:, h, :],
                                rhs=Es[par][jt],
                                start=(jt == 0),
                                stop=(jt == 3),
                            )
                        rs = spool.tile([1, NB], F32, name="rs", tag="rs")
                        nc.vector.reciprocal(out=rs, in_=avp[DH:DH + 1, :])
                        bc = spool.tile([64, NB], F32, name="bc", tag="bc")
                        nc.gpsimd.partition_broadcast(bc, rs)
                        if off == 0:
                            nc.vector.tensor_mul(
                                out=o_sb[mt][0:64, :], in0=avp[0:DH, :], in1=bc
                            )
                        else:
                            # DVE lanes cannot shift partitions; bounce via DMA.
                            tmp = spool.tile([64, NB], BF16, name="tmp", tag="tmp")
                            nc.vector.tensor_mul(out=tmp, in0=avp[0:DH, :], in1=bc)
                            nc.sync.dma_start(out=o_sb[mt][64:128, :], in_=tmp)

                # ---- output projection back to natural [t, d] ----
                for tt in range(4):
                    yb = ypool.tile([128, D], F32, name="yb", tag="yb")
                    if pair:
                        fps2 = [
                            av_ps.tile([128, NB], F32, name="fps", tag="avp")
                            for _ in range(2)
                        ]
                        for mt in range(8):
                            for half in range(2):
                                nc.tensor.matmul(
                                    fps2[half],
                                    lhsT=o_sb[mt][:, tt * 128:(tt + 1) * 128],
                                    rhs=wo_sb[mt][:, half * NB:(half + 1) * NB],
                                    start=(mt == 0),
                                    stop=(mt == 7),
                                )
                        for half in range(2):
                            nc.scalar.copy(
                                out=yb[:, half * NB:(half + 1) * NB], in_=fps2[half]
                            )
                    else:
                        for half in range(2):
                            fps = av_ps.tile([128, NB], F32, name="fps", tag="avp")
                            for mt in range(8):
                                nc.tensor.matmul(
                                    fps,
                                    lhsT=o_sb[mt][:, tt * 128:(tt + 1) * 128],
                                    rhs=wo_sb[mt][:, half * NB:(half + 1) * NB],
                                    start=(mt == 0),
                                    stop=(mt == 7),
                                )
                            nc.scalar.copy(out=yb[:, half * NB:(half + 1) * NB], in_=fps)
                    nc.sync.dma_start(
                        out=y[c * NB + tt * 128:c * NB + (tt + 1) * 128, :], in_=yb
                    )

    nc.compile()
    return nc


_cache = threading.Lock()
_built = {}


def get_bass(repeat=1, loop=1, loads_in_loop=True, pair=False):
    with _cache:
        key = (repeat, loop, loads_in_loop, pair)
        if key not in _built:
            _built[key] = _build_bass(repeat, loop, loads_in_loop, pair)
        return _built[key]


def make_in_maps(x, w_qkv, w_out, mask):
    """Build the 8 per-core input dicts from full inputs."""
    x = np.asarray(x, dtype=np.float32)
    w_qkv = np.asarray(w_qkv, dtype=np.float32)
    w_out = np.asarray(w_out, dtype=np.float32)
    mask = np.asarray(mask)

    wqkvT = np.ascontiguousarray(w_qkv.T).astype(BF16NP)      # [D, 3*inner]
    woutT = np.ascontiguousarray(w_out.T).astype(BF16NP)      # [inner, D]

    # (1 - eye) tiles in the sim^T [j, i] layout: ec[p, jt, i] = 0 iff jt*128+p == i
    jidx = (np.arange(4)[:, None] * 128 + np.arange(128)[None, :])  # [jt, p] -> j
    ec = np.ones((128, 4, NB), np.float32)
    for jt in range(4):
        ec[np.arange(128), jt, jidx[jt]] = 0.0
    ec = ec.astype(BF16NP)

    xr = x.reshape(B, F, NB, D)
    maskr = mask.reshape(B, HEADS, F, NB)

    in_maps = []
    for core in range(NCORES):
        chunks = (2 * core, 2 * core + 1)
        xc = np.concatenate([xr[g // F, g % F] for g in chunks], axis=0)  # [TPC, D]
        xT = np.ascontiguousarray(xc.T).astype(BF16NP)                    # [D, TPC]
        mb = np.zeros((CPC, HEADS, 4, 128), np.float32)
        for ci, g in enumerate(chunks):
            mrow = maskr[g // F, :, g % F, :]                             # [HEADS, NB]
            mb[ci] = np.where(mrow.reshape(HEADS, 4, 128) == 0, MASK_NEG, 0.0)
        maskT = np.ascontiguousarray(mb.transpose(3, 0, 1, 2))            # [128, CPC, HEADS, 4]
        in_maps.append({
            "xT": xT,
            "wqkvT": wqkvT,
            "woutT": woutT,
            "maskT": maskT,
            "eyec": ec,
        })
    return in_maps


def assemble_output(results, b_out):
    """Stitch the 8 per-core [TPC, D] outputs back into [B, N, D]."""
    y = np.empty((B, N, D), np.float32)
    for core in range(NCORES):
        yc = results[core]["y"]
        for ci, g in enumerate((2 * core, 2 * core + 1)):
            b, fi = g // F, g % F
            y[b, fi * NB:(fi + 1) * NB, :] = yc[ci * NB:(ci + 1) * NB, :]
    y += np.asarray(b_out, dtype=np.float32)[None, None, :]
    return y


def run(x, w_qkv, w_out, b_out, mask, trace=False, **spmd_kwargs):
    from concourse.bass_utils import run_bass_kernel_spmd

    nc = get_bass()
    in_maps = make_in_maps(x, w_qkv, w_out, mask)
    res = run_bass_kernel_spmd(
        nc, in_maps, core_ids=list(range(NCORES)), trace=trace, **spmd_kwargs
    )
    return assemble_output(res.results, b_out), res


def kernel(x, w_qkv, w_out, b_out, mask, f, diag):
    assert int(f) == F and int(diag) == 1, (f, diag)
    out, _ = run(x, w_qkv, w_out, b_out, mask)
    return out
